# revision 1
# baseline (speedup 1.0000x reference)
"""3-layer GAT on 8 Trainium2 NeuronCores.

Strategy (edge-parallel by destination):
- Nodes are split across 8 cores by original id (6250/core), then degree-
  balanced into 49 tiles of 128 "slots" per core (host permutation).
- Per layer, a node table T[v] = [el(v) | ft(v)] lives in DRAM on every core
  (built distributedly, then AllGather'ed).  ft = h @ W, el/er come from
  host-folded weight columns (el = h @ (W*a_l folded)).
- Each core processes only edges whose dst is in its slice, grouped by dst
  tile.  Per tile: dma_gather pulls the 768B/256B table rows of the edge
  sources (int16 indices -> table split in two 25088-row halves), a one-hot
  matrix (DVE is_equal vs iota) turns the segment-softmax numerator and
  denominator into PE matmuls accumulated in PSUM, er[dst] is expanded
  edge-wise with a K=1 ones-outer-product + is_equal + small matmuls.
- exp() is the only ScalarE activation used (leaky-relu runs on DVE), so the
  activation table never swaps.
- Softmax shift invariance removes the segment-max pass entirely (logits are
  O(1) by construction).
"""

import numpy as np
import ml_dtypes

N_NODES = 50000
N_EDGES = 800000
IN_FEATS = 128
HID = 32
HEADS = 8
OUT_FEATS = 32
NEG_SLOPE = 0.2

NC_N = 8                 # cores
NPC = N_NODES // NC_N    # real nodes per core (6250)
NT = 49                  # dst tiles per core
SLOTS = NT * 128         # 6272 slots per core
HALF = 4 * SLOTS         # 25088 table rows per half
VTOT = NC_N * SLOTS      # 50176 table rows

BF16 = ml_dtypes.bfloat16

_CACHE = {}
_last_in_maps = None


# ----------------------------------------------------------------------------
# Host-side graph preparation
# ----------------------------------------------------------------------------

def _prep_graph(src, dst):
    """Partition edges by dst core, degree-balance nodes into tiles, build all
    per-core index tensors.  Returns dict of host arrays + CPH."""
    src = np.asarray(src).astype(np.int64)
    dst = np.asarray(dst).astype(np.int64)

    ecore = dst // NPC

    # --- per-core node -> (tile, pos) assignment, degree balanced (LPT) ---
    slot_g = np.zeros(N_NODES, dtype=np.int64)      # local slot (0..SLOTS)
    degA = np.zeros(N_NODES, dtype=np.int64)
    degB = np.zeros(N_NODES, dtype=np.int64)
    half_e = (src >= 4 * NPC).astype(np.int64)   # src >= 25000 -> half B
    np.add.at(degA, dst[half_e == 0], 1)
    np.add.at(degB, dst[half_e == 1], 1)

    for k in range(NC_N):
        lo, hi = k * NPC, (k + 1) * NPC
        nodes = np.arange(lo, hi)
        d = degA[lo:hi] + degB[lo:hi]
        order = np.argsort(-d, kind="stable")
        loads = np.zeros(NT, dtype=np.int64)
        counts = np.zeros(NT, dtype=np.int64)
        tile_of = np.zeros(NPC, dtype=np.int64)
        pos_of = np.zeros(NPC, dtype=np.int64)
        # greedy: put next-heaviest node on lightest non-full tile
        for i in order:
            t = np.argmin(np.where(counts < 128, loads, np.iinfo(np.int64).max))
            tile_of[i] = t
            pos_of[i] = counts[t]
            counts[t] += 1
            loads[t] += d[i]
        slot_g[nodes] = tile_of * 128 + pos_of

    srcslot = (src // NPC) * SLOTS + slot_g[src]    # global table row of src
    dslot = slot_g[dst]                              # local slot of dst
    dtile = dslot // 128
    dstl = dslot % 128

    # --- group edges by (core, tile, half) ---
    key = (ecore * NT + dtile) * 2 + half_e
    order = np.argsort(key, kind="stable")
    key_s = key[order]
    ngroups = NC_N * NT * 2
    counts = np.bincount(key_s, minlength=ngroups)
    starts = np.concatenate([[0], np.cumsum(counts)[:-1]])
    j_within = np.arange(len(src)) - starts[key_s]

    CPH = int(np.ceil(counts.max() / 128))
    CAP = CPH * 128

    gidx = np.zeros((NC_N, NT, 2, CAP), dtype=np.int16)
    dstl_a = np.full((NC_N, NT, 2, CAP), -1.0, dtype=np.float32)

    ks = key_s
    gidx[ks // (NT * 2), (ks // 2) % NT, ks % 2, j_within] = (
        srcslot[order] - (ks % 2) * HALF
    ).astype(np.int16)
    dstl_a[ks // (NT * 2), (ks // 2) % NT, ks % 2, j_within] = dstl[order]

    # --- per-core tensors ---
    CPT = 2 * CPH
    TSLOT = CPT * 128
    idxA, idxB, dstlT, dstlF = [], [], [], []
    for k in range(NC_N):
        ia = gidx[k, :, 0, :].reshape(-1)            # [NT*CAP]
        ib = gidx[k, :, 1, :].reshape(-1)
        wrapA = np.tile(ia.reshape(-1, 16).T, (8, 1))
        wrapB = np.tile(ib.reshape(-1, 16).T, (8, 1))
        idxA.append(np.ascontiguousarray(wrapA))
        idxB.append(np.ascontiguousarray(wrapB))
        # dstlT[p, t*CPT + c] = dstl of slot (c*128+p) in tile t
        # dstl_a[k, t, h, j]: j = c_h*128 + p; chunk c = h*CPH + c_h
        dT2 = dstl_a[k].reshape(NT, 2, CPH, 128)     # [t, h, c_h, p]
        dT2 = dT2.reshape(NT, CPT, 128)              # [t, c, p]
        dstlT.append(np.ascontiguousarray(
            dT2.transpose(2, 0, 1).reshape(128, NT * CPT).astype(BF16)))
        dstlF.append(np.ascontiguousarray(dT2.reshape(NT, TSLOT).astype(BF16)))

    return {
        "CPH": CPH,
        "slot_g": slot_g,
        "idxA": idxA, "idxB": idxB,
        "dstlT": dstlT, "dstlF": dstlF,
    }


def _fold_w(W, al, ar):
    """Wc = [Wl | W | Wr]: el = h@Wl, ft = h@W, er = h@Wr."""
    Din = W.shape[0]
    H, C = al.shape
    W3 = W.reshape(Din, H, C)
    Wl = np.einsum("dhc,hc->dh", W3, al)
    Wr = np.einsum("dhc,hc->dh", W3, ar)
    return np.concatenate([Wl, W, Wr], axis=1).astype(BF16)  # [Din, H + H*C + H]


# ----------------------------------------------------------------------------
# Device program
# ----------------------------------------------------------------------------

def _build_program(CPH, stages=None):
    import concourse.bass as bass
    import concourse.mybir as mybir
    import concourse.tile as tile
    from concourse import bacc
    from concourse.masks import make_identity

    f32 = mybir.dt.float32
    bf16 = mybir.dt.bfloat16
    i16 = mybir.dt.int16
    Alu = mybir.AluOpType
    Act = mybir.ActivationFunctionType

    CPT = 2 * CPH
    TSLOT = CPT * 128
    ICOL = NT * CPH * 8          # idx cols per half: NT*CPH*128/16

    nc = bacc.Bacc("TRN2", target_bir_lowering=False, debug=False,
                   num_devices=NC_N, num_swdge_queues=4)

    # ---- I/O ----
    xT = nc.dram_tensor("xT", [128, SLOTS], bf16, kind="ExternalInput")
    wc0 = nc.dram_tensor("wc0", [128, 272], bf16, kind="ExternalInput")
    wc1 = nc.dram_tensor("wc1", [256, 272], bf16, kind="ExternalInput")
    wc2 = nc.dram_tensor("wc2", [256, 34], bf16, kind="ExternalInput")
    idxA = nc.dram_tensor("idxA", [128, ICOL], i16, kind="ExternalInput")
    idxB = nc.dram_tensor("idxB", [128, ICOL], i16, kind="ExternalInput")
    dstlT = nc.dram_tensor("dstlT", [128, NT * CPT], bf16, kind="ExternalInput")
    dstlF = nc.dram_tensor("dstlF", [NT, TSLOT], bf16, kind="ExternalInput")
    iota128 = nc.dram_tensor("iota128", [128, 128], bf16, kind="ExternalInput")
    iotaP = nc.dram_tensor("iotaP", [128, 1], f32, kind="ExternalInput")
    ones128 = nc.dram_tensor("ones128", [128, 128], bf16, kind="ExternalInput")
    ones8 = nc.dram_tensor("ones8", [128, 8], f32, kind="ExternalInput")
    out_d = nc.dram_tensor("out", [SLOTS, 32], f32, kind="ExternalOutput")

    # ---- internal DRAM (tables) ----
    ohD = nc.dram_tensor("ohD", [128, NT * CPH * 2 * 128], bf16)
    ohTD = nc.dram_tensor("ohTD", [128, NT * CPH * 2 * 128], bf16)
    T0s = nc.dram_tensor("T0s", [SLOTS, 384], bf16)
    T1s = nc.dram_tensor("T1s", [SLOTS, 384], bf16)
    T2s = nc.dram_tensor("T2s", [SLOTS, 128], bf16)
    T0f = nc.dram_tensor("T0f", [VTOT, 384], bf16, addr_space="Shared")
    T1f = nc.dram_tensor("T1f", [VTOT, 384], bf16, addr_space="Shared")
    T2f = nc.dram_tensor("T2f", [VTOT, 128], bf16, addr_space="Shared")

    def bc(ap, dims):
        """Rebuild AP with explicit [step, count] free dims."""
        return bass.AP(ap.tensor, ap.offset, [ap.ap[0]] + dims)

    with tile.TileContext(nc) as tc:
        import contextlib
        ctx = contextlib.ExitStack()
        with ctx:
            consts = ctx.enter_context(tc.tile_pool(name="consts", bufs=1))
            persist = ctx.enter_context(tc.tile_pool(name="persist", bufs=1))
            gpool = ctx.enter_context(tc.tile_pool(name="gather", bufs=5))
            mpool = ctx.enter_context(tc.tile_pool(name="msg", bufs=2))
            opool = ctx.enter_context(tc.tile_pool(name="oneh", bufs=3))
            spool = ctx.enter_context(tc.tile_pool(name="small", bufs=3))
            tpool = ctx.enter_context(tc.tile_pool(name="tbuild", bufs=2))
            pp_tb = ctx.enter_context(tc.tile_pool(name="ps_tb", bufs=1, space="PSUM"))
            pp_agg = ctx.enter_context(tc.tile_pool(name="ps_agg", bufs=2, space="PSUM"))
            ppr2 = ctx.enter_context(tc.tile_pool(name="ps_rep2", bufs=1, space="PSUM"))
            pp_tp = ctx.enter_context(tc.tile_pool(name="ps_tp", bufs=1, space="PSUM"))

            # ---- load constants ----
            iota128_sb = consts.tile([128, 128], bf16)
            nc.sync.dma_start(out=iota128_sb[:], in_=iota128[:, :])
            iotaP_sb = consts.tile([128, 1], f32)
            nc.sync.dma_start(out=iotaP_sb[:], in_=iotaP[:, :])
            ones128_sb = consts.tile([128, 128], bf16)
            nc.sync.dma_start(out=ones128_sb[:], in_=ones128[:, :])
            ones8_sb = consts.tile([128, 8], f32)
            nc.sync.dma_start(out=ones8_sb[:], in_=ones8[:, :])
            ident = consts.tile([128, 128], bf16)
            make_identity(nc, ident[:])
            zer_bf = consts.tile([128, 1], bf16)
            nc.gpsimd.memset(zer_bf[:], 0)
            zer_f32 = consts.tile([128, 1], f32)
            nc.gpsimd.memset(zer_f32[:], 0)
            slope_bf = consts.tile([128, 1], bf16)
            nc.gpsimd.memset(slope_bf[:], NEG_SLOPE)
            eps_f32 = consts.tile([128, 1], f32)
            nc.gpsimd.memset(eps_f32[:], 1e-30)
            zpad = consts.tile([128, 128], bf16)
            nc.gpsimd.memset(zpad[:], 0)

            idxA_sb = persist.tile([128, ICOL], i16)
            nc.sync.dma_start(out=idxA_sb[:], in_=idxA[:, :])
            idxB_sb = persist.tile([128, ICOL], i16)
            nc.sync.dma_start(out=idxB_sb[:], in_=idxB[:, :])
            dstlT_sb = persist.tile([128, NT * CPT], bf16)
            nc.sync.dma_start(out=dstlT_sb[:], in_=dstlT[:, :])
            xT_sb = persist.tile([128, SLOTS], bf16)
            nc.sync.dma_start(out=xT_sb[:], in_=xT[:, :])
            wc0_sb = persist.tile([128, 272], bf16)
            nc.sync.dma_start(out=wc0_sb[:], in_=wc0[:, :])
            wc1_sb = persist.tile([128, 2, 272], bf16)
            nc.sync.dma_start(out=wc1_sb[:, 0, :], in_=wc1[0:128, :])
            nc.sync.dma_start(out=wc1_sb[:, 1, :], in_=wc1[128:256, :])
            wc2_sb = persist.tile([128, 2, 34], bf16)
            nc.sync.dma_start(out=wc2_sb[:, 0, :], in_=wc2[0:128, :])
            nc.sync.dma_start(out=wc2_sb[:, 1, :], in_=wc2[128:256, :])

            er_sb = persist.tile([128, NT, 8], bf16, tag="er")
            tsbA = persist.tile([128, 384], bf16, tag="tsbA")
            tsbB = persist.tile([128, 384], bf16, tag="tsbB")
            nc.gpsimd.memset(tsbA[:], 0)
            nc.gpsimd.memset(tsbB[:], 0)
            hT_sb = persist.tile([128, 2, SLOTS], bf16, tag="hT")

            def build_onehots():
                for t in range(NT):
                    oh = opool.tile([128, CPT, 128], bf16, tag="oh")
                    dT = dstlT_sb[:, t * CPT:(t + 1) * CPT]
                    nc.vector.tensor_tensor(
                        out=oh[:],
                        in0=bc(dT, [[dT.ap[1][0], CPT], [0, 128]]),
                        in1=bc(iota128_sb[:], [[0, CPT], [1, 128]]),
                        op=Alu.is_equal)
                    nc.sync.dma_start(
                        out=ohD[:, t * TSLOT:(t + 1) * TSLOT],
                        in_=oh[:].rearrange("p c e -> p (c e)"))
                    QS = 1152
                    ohT = opool.tile([128, CPT, 128], bf16, tag="ohT")
                    stage = spool.tile([1, TSLOT], bf16, tag="stage")
                    nc.sync.dma_start(out=stage[:], in_=dstlF[t:t + 1, :])
                    ohT_flat = ohT[:].rearrange("p c e -> p (c e)")
                    for s0 in range(0, TSLOT, QS):
                        s1 = s0 + QS
                        rep = ppr2.tile([128, QS], f32, tag="rep", space="PSUM")
                        for q0 in range(0, QS, 512):
                            q1 = min(q0 + 512, QS)
                            nc.tensor.matmul(out=rep[:, q0:q1],
                                             lhsT=ones128_sb[0:1, :],
                                             rhs=stage[:, s0 + q0:s0 + q1],
                                             start=True, stop=True)
                        nc.vector.tensor_tensor(
                            out=ohT_flat[:, s0:s1],
                            in0=bc(iotaP_sb[:, 0:1], [[0, QS]]),
                            in1=rep[:],
                            op=Alu.is_equal)
                    nc.sync.dma_start(
                        out=ohTD[:, t * TSLOT:(t + 1) * TSLOT],
                        in_=ohT_flat)

            qn = [0]

            def next_q():
                qn[0] = (qn[0] + 1) % 4
                return qn[0]

            # ---------------- table build ----------------
            def build_table(layer, Ts, ELW, FT):
                """Ts rows = [el f32 (ELW) | ft bf16 (FT) | pad].  Also fills
                er_sb.  lhsT: layer0 -> xT_sb (K=128); else hT_sb (K=256)."""
                ROW = 384 if FT == 256 else 128
                for t in range(NT):
                    ps = pp_tb.tile([128, ELW + FT + ELW], f32, tag="tb_ps",
                                    space="PSUM")
                    if layer == 0:
                        nc.tensor.matmul(out=ps[:], lhsT=xT_sb[:, t * 128:(t + 1) * 128],
                                         rhs=wc0_sb[:], start=True, stop=True)
                    else:
                        w = wc1_sb if layer == 1 else wc2_sb
                        for kb in range(2):
                            nc.tensor.matmul(out=ps[:],
                                             lhsT=hT_sb[:, kb, t * 128:(t + 1) * 128],
                                             rhs=w[:, kb, :],
                                             start=(kb == 0), stop=(kb == 1))
                    tsb = (tsbA if t % 2 == 0 else tsbB)[:, 0:ROW]
                    nc.vector.tensor_copy(
                        out=tsb[:, 0:2 * ELW].bitcast(f32), in_=ps[:, 0:ELW])
                    nc.vector.tensor_copy(
                        out=tsb[:, 2 * ELW:2 * ELW + FT], in_=ps[:, ELW:ELW + FT])
                    nc.vector.tensor_copy(
                        out=er_sb[:, t, 0:ELW], in_=ps[:, ELW + FT:ELW + FT + ELW])
                    nc.sync.dma_start(out=Ts[t * 128:(t + 1) * 128, :], in_=tsb[:])

            def allgather(Ts, Tf):
                nc.gpsimd.collective_compute(
                    "AllGather", Alu.bypass,
                    replica_groups=[list(range(NC_N))],
                    ins=[Ts[:, :]], outs=[Tf[:, :]],
                )

            # ---------------- edge phase ----------------
            def edge_phase(layer, Tf, ELW, H, FT):
                ROW = 384 if FT == 256 else 128
                MSGN = FT + 1 if H == 1 else FT + H
                ftc0 = 2 * ELW                      # first ft col in row
                for t in range(NT):
                    # gathers (A/B halves)
                    gA = gpool.tile([128, CPH, ROW], bf16, tag="gA")
                    gB = gpool.tile([128, CPH, ROW], bf16, tag="gB")
                    cols = CPH * 8
                    nc.gpsimd.dma_gather(
                        out_ap=gA[:], in_ap=Tf[0:HALF, :],
                        idxs_ap=idxA_sb[:, t * cols:(t + 1) * cols],
                        num_idxs=CPH * 128, num_idxs_reg=CPH * 128,
                        elem_size=ROW, single_packet=False, queue_num=next_q())
                    nc.gpsimd.dma_gather(
                        out_ap=gB[:], in_ap=Tf[HALF:2 * HALF, :],
                        idxs_ap=idxB_sb[:, t * cols:(t + 1) * cols],
                        num_idxs=CPH * 128, num_idxs_reg=CPH * 128,
                        elem_size=ROW, single_packet=False, queue_num=next_q())

                    # stream precomputed one-hots
                    oh = opool.tile([128, CPT, 128], bf16, tag="oh")
                    nc.sync.dma_start(out=oh[:].rearrange("p c e -> p (c e)"),
                                      in_=ohD[:, t * TSLOT:(t + 1) * TSLOT])
                    ohT = opool.tile([128, CPT, 128], bf16, tag="ohT")
                    nc.sync.dma_start(out=ohT[:].rearrange("p c e -> p (c e)"),
                                      in_=ohTD[:, t * TSLOT:(t + 1) * TSLOT])

                    # er expansion: er_d[e, h] per chunk
                    erd = pp_tb.tile([128, CPT * H], f32, tag="tb_ps",
                                     space="PSUM")
                    for c in range(CPT):
                        nc.tensor.matmul(out=erd[:, c * H:(c + 1) * H],
                                         lhsT=ohT[:, c, :],
                                         rhs=er_sb[:, t, 0:H],
                                         start=True, stop=True)

                    # logits z = el + er_d ; p = exp(lrelu(z))
                    z = spool.tile([128, CPT, H], bf16, tag="z")
                    for hf, g in ((0, gA), (1, gB)):
                        nc.vector.tensor_tensor(
                            out=z[:, hf * CPH:(hf + 1) * CPH, :],
                            in0=g[:, :, 0:2 * ELW].bitcast(f32),
                            in1=erd[:, hf * CPH * H:(hf + 1) * CPH * H]
                                .rearrange("p (c h) -> p c h", c=CPH),
                            op=Alu.add)
                    zf = z[:].rearrange("p c h -> p (c h)")
                    zs = spool.tile([128, CPT * H], bf16, tag="zs")
                    nc.vector.tensor_tensor(out=zs[:], in0=zf,
                                            in1=bc(slope_bf[:, 0:1], [[0, CPT * H]]),
                                            op=Alu.mult)
                    nc.vector.tensor_tensor(out=zs[:], in0=zf, in1=zs[:],
                                            op=Alu.max)
                    p = spool.tile([128, CPT, H], bf16, tag="p")
                    nc.scalar.activation(
                        out=p[:].rearrange("p c h -> p (c h)"), in_=zs[:],
                        func=Act.Exp)

                    # messages: [p*ft | p]
                    msg = mpool.tile([128, CPT, MSGN], bf16, tag="msg")
                    for hf, g in ((0, gA), (1, gB)):
                        psl = p[:, hf * CPH:(hf + 1) * CPH, :]
                        nc.vector.tensor_tensor(
                            out=bc(msg[:, hf * CPH:(hf + 1) * CPH, 0:FT],
                                   [[MSGN, CPH], [32, H], [1, 32]]),
                            in0=bc(g[:, :, ftc0:ftc0 + FT],
                                   [[ROW, CPH], [32, H], [1, 32]]),
                            in1=bc(psl, [[H, CPH], [1, H], [0, 32]]),
                            op=Alu.mult)
                    nc.vector.tensor_copy(out=msg[:, :, FT:FT + H],
                                          in_=p[:, :, :])

                    # segment reduction
                    agg = pp_agg.tile([128, MSGN], f32, tag="agg",
                                      space="PSUM")
                    for c in range(CPT):
                        nc.tensor.matmul(out=agg[:], lhsT=oh[:, c, :],
                                         rhs=msg[:, c, :],
                                         start=(c == 0), stop=(c == CPT - 1))

                    # epilogue
                    s_sb = spool.tile([128, H], f32, tag="s")
                    nc.vector.tensor_tensor(out=s_sb[:], in0=agg[:, FT:FT + H],
                                            in1=bc(eps_f32[:, 0:1], [[0, H]]),
                                            op=Alu.add)
                    rs = spool.tile([128, H], f32, tag="rs")
                    nc.vector.reciprocal(out=rs[:], in_=s_sb[:])
                    if layer < 2:
                        h_t = spool.tile([128, 256], bf16, tag="ht")
                        nc.vector.tensor_tensor(
                            out=bc(h_t[:], [[32, H], [1, 32]]),
                            in0=bc(agg[:, 0:FT], [[32, H], [1, 32]]),
                            in1=bc(rs[:], [[1, H], [0, 32]]),
                            op=Alu.mult)
                        nc.vector.tensor_tensor(out=h_t[:], in0=h_t[:],
                                                in1=bc(zer_bf[:, 0:1], [[0, 256]]),
                                                op=Alu.max)
                        # transpose into hT_sb
                        for b in range(2):
                            tp = pp_tp.tile([128, 128], bf16, tag="tp",
                                            space="PSUM")
                            nc.tensor.transpose(out=tp[:],
                                                in_=h_t[:, b * 128:(b + 1) * 128],
                                                identity=ident[:])
                            nc.vector.tensor_copy(
                                out=hT_sb[:, b, t * 128:(t + 1) * 128], in_=tp[:])
                    else:
                        o1 = spool.tile([128, 32], f32, tag="o1")
                        nc.vector.tensor_tensor(
                            out=o1[:], in0=agg[:, 0:32],
                            in1=bc(rs[:, 0:1], [[0, 32]]), op=Alu.mult)
                        nc.vector.tensor_tensor(out=o1[:], in0=o1[:],
                                                in1=bc(zer_f32[:, 0:1], [[0, 32]]),
                                                op=Alu.max)
                        nc.scalar.activation(out=o1[:], in_=o1[:], func=Act.Exp)
                        ssum = spool.tile([128, 1], f32, tag="ssum")
                        nc.vector.tensor_reduce(out=ssum[:], in_=o1[:],
                                                axis=mybir.AxisListType.X,
                                                op=Alu.add)
                        rr = spool.tile([128, 1], f32, tag="rr")
                        nc.vector.reciprocal(out=rr[:], in_=ssum[:])
                        ofin = spool.tile([128, 32], f32, tag="ofin")
                        nc.vector.tensor_tensor(out=ofin[:], in0=o1[:],
                                                in1=bc(rr[:], [[0, 32]]),
                                                op=Alu.mult)
                        nc.sync.dma_start(out=out_d[t * 128:(t + 1) * 128, :],
                                          in_=ofin[:])

            # ================= schedule =================
            all_stages = ["t0", "ag0", "e0", "t1", "ag1", "e1", "t2", "ag2", "e2"]
            st = all_stages if stages is None else stages
            build_onehots()
            if "t0" in st:
                build_table(0, T0s, 8, 256)
            if "ag0" in st:
                allgather(T0s, T0f)
            if "e0" in st:
                edge_phase(0, T0f, 8, 8, 256)
            if "t1" in st:
                build_table(1, T1s, 8, 256)
            if "ag1" in st:
                allgather(T1s, T1f)
            if "e1" in st:
                edge_phase(1, T1f, 8, 8, 256)
            if "t2" in st:
                build_table(2, T2s, 1, 32)
            if "ag2" in st:
                allgather(T2s, T2f)
            if "e2" in st:
                edge_phase(2, T2f, 1, 1, 32)

    nc.compile()
    return nc


# ----------------------------------------------------------------------------
# Entry point
# ----------------------------------------------------------------------------

def kernel(x, src, dst, W0, al0, ar0, b0, W1, al1, ar1, b1, W2, al2, ar2, b2):
    from concourse.bass_utils import run_bass_kernel_spmd

    x = np.asarray(x, dtype=np.float32)
    g = _prep_graph(src, dst)
    CPH = g["CPH"]

    key = ("prog", CPH)
    if key not in _CACHE:
        _CACHE[key] = _build_program(CPH)
    nc = _CACHE[key]

    wc0 = _fold_w(np.asarray(W0, np.float32), np.asarray(al0, np.float32),
                  np.asarray(ar0, np.float32))
    wc1 = _fold_w(np.asarray(W1, np.float32), np.asarray(al1, np.float32),
                  np.asarray(ar1, np.float32))
    wc2 = _fold_w(np.asarray(W2, np.float32), np.asarray(al2, np.float32),
                  np.asarray(ar2, np.float32))

    iota128 = np.broadcast_to(np.arange(128, dtype=np.float32), (128, 128)) \
        .astype(BF16)
    iotaP = np.arange(128, dtype=np.float32).reshape(128, 1)
    ones128 = np.ones((128, 128), dtype=BF16)
    ones8 = np.ones((128, 8), dtype=np.float32)

    slot_g = g["slot_g"]
    in_maps = []
    for k in range(NC_N):
        lo, hi = k * NPC, (k + 1) * NPC
        xTk = np.zeros((128, SLOTS), dtype=BF16)
        xTk[:, slot_g[lo:hi]] = x[lo:hi].T.astype(BF16)
        in_maps.append({
            "xT": np.ascontiguousarray(xTk),
            "wc0": wc0, "wc1": wc1, "wc2": wc2,
            "idxA": g["idxA"][k], "idxB": g["idxB"][k],
            "dstlT": g["dstlT"][k], "dstlF": g["dstlF"][k],
            "iota128": iota128, "iotaP": iotaP,
            "ones128": ones128, "ones8": ones8,
        })

    global _last_in_maps
    _last_in_maps = in_maps
    res = run_bass_kernel_spmd(nc, in_maps, core_ids=list(range(NC_N)))

    out = np.empty((N_NODES, 32), dtype=np.float32)
    for k in range(NC_N):
        lo, hi = k * NPC, (k + 1) * NPC
        out[lo:hi] = res.results[k]["out"][slot_g[lo:hi]]
    return out



# revision 10
# speedup vs baseline: 3.9586x; 3.9586x over previous
"""3-layer GAT on 8 Trainium2 NeuronCores — lane-aligned edge layout.

Strategy (v2, replaces one-hot/matmul scatter of the baseline):
- Every dst node owns a fixed (tile, lane) slot; chunk c of lane p holds the
  c-th in-edge of node p.  The segment-sum then becomes PSUM accumulation of
  msg chunks through an identity-lhsT matmul, and er[dst] expansion is a
  stride-0 broadcast read — the one-hot matrices, their DRAM streaming
  (231MB), and the er-expansion matmuls all disappear.
- dma_gather idx are int16, so the node table is split in two halves (E/O).
  A greedy 2-coloring of SRC nodes balances each dst's in-edge colors, then
  nodes are 2D snake-binned by (degE, degO) and dealt to cores by global
  rank, keeping per-tile max degrees tight (Sum maxE+maxO ~= 906 chunks vs
  882 for the baseline's free assignment) and identical across cores.
- Padding slots gather a dummy table row whose el is -1e4, so exp() gives
  exactly 0 — no masks needed.
- The next layer's table build is fused into each tile's epilogue; exp is
  the only DVE-expensive activation left (lrelu runs as one
  scalar_tensor_tensor, relu + softmax epilogue on the scalar engine).
"""

import numpy as np
import ml_dtypes

N_NODES = 50000
N_EDGES = 800000
IN_FEATS = 128
HID = 32
HEADS = 8
OUT_FEATS = 32
NEG_SLOPE = 0.2

NC_N = 8                  # cores
NT = 49                   # dst tiles per core
SLOTS = NT * 128          # 6272 slots per core
HSLOT = SLOTS // 2        # 3136 slots per half
HROWS = NC_N * HSLOT      # 25088 table rows per half
ROW01 = 384               # bf16 cols per table row, layers 0/1 (768B)
ROW2 = 128                # layer 2 (256B)
DUM_E = 3135              # local slot of the dummy row (E half, all cores)
DUM_O = 6271              # local slot of the dummy row (O half)

BF16 = ml_dtypes.bfloat16

_CACHE = {}
_last_in_maps = None


# ----------------------------------------------------------------------------
# Host-side graph preparation
# ----------------------------------------------------------------------------

def _color_sources(src, dst):
    """Greedy 2-coloring of nodes minimizing per-dst color imbalance."""
    odeg = np.bincount(src, minlength=N_NODES)
    osort = np.argsort(src, kind="stable")
    dst_by_src = dst[osort]
    optr = np.concatenate([[0], np.cumsum(odeg)])
    order = np.argsort(-odeg, kind="stable")
    imb = np.zeros(N_NODES, dtype=np.int64)
    color = np.zeros(N_NODES, dtype=np.int8)
    for v in order:
        dd = dst_by_src[optr[v]:optr[v + 1]]
        c = -1 if imb[dd].sum() > 0 else 1
        color[v] = c
        imb[dd] += c
    return color


def _prep_graph(src, dst):
    src = np.asarray(src).astype(np.int64)
    dst = np.asarray(dst).astype(np.int64)
    deg = np.bincount(dst, minlength=N_NODES)

    color = _color_sources(src, dst)
    # cap each color's population so every core keeps >=1 fake slot per half
    for cval in (1, -1):
        m = np.where(color == cval)[0]
        excess = len(m) - (HSLOT - 1) * NC_N
        if excess > 0:
            flip = m[np.argsort(deg[m])][:excess]
            color[flip] = -cval

    degE = np.bincount(dst[color[src] == 1], minlength=N_NODES)
    degO = deg - degE

    # 2D snake order by (degE, degO); E desc, O asc so the middle tile mixes
    # the low-degree tails of both halves.
    def snake(nodes):
        dE, dO = degE[nodes], degO[nodes]
        dOk = np.where(dE % 2 == 0, dO, 10 ** 6 - dO)
        return nodes[np.argsort(-(dE * np.int64(10 ** 7) + dOk), kind="stable")]

    Em = snake(np.where(color == 1)[0])
    Om = snake(np.where(color == -1)[0])[::-1]

    node_core = np.zeros(N_NODES, dtype=np.int64)
    slot = np.full(N_NODES, -1, dtype=np.int64)
    rE = np.arange(len(Em))
    node_core[Em] = rE % NC_N
    slot[Em] = rE // NC_N
    rO = np.arange(len(Om))
    node_core[Om] = rO % NC_N
    slot[Om] = HSLOT + rO // NC_N
    assert slot[Em].max() < DUM_E and slot[Om].max() < DUM_O

    isE = color == 1
    row_half = np.where(isE, node_core * HSLOT + slot,
                        node_core * HSLOT + slot - HSLOT)

    ecore = node_core[dst]
    etile = slot[dst] // 128
    elane = slot[dst] % 128
    egrp = isE[src]          # True -> E gather

    # per (core, tile, grp, lane) sequence numbers
    key = ((ecore * NT + etile) * 2 + (~egrp)) * 128 + elane
    order = np.argsort(key, kind="stable")
    key_s = key[order]
    starts = np.concatenate([[0], np.cumsum(np.bincount(
        key_s, minlength=NC_N * NT * 2 * 128))[:-1]])
    seq = np.empty(len(src), dtype=np.int64)
    seq[order] = np.arange(len(src)) - starts[key_s]

    # chunk counts per (core, tile, grp) -> cross-core max
    cnt = np.zeros((NC_N, NT, 2, 128), dtype=np.int64)
    np.add.at(cnt, (ecore, etile, (~egrp).astype(np.int64), elane), 1)
    CEa = cnt[:, :, 0, :].max(axis=(0, 2))      # [NT]
    COa = cnt[:, :, 1, :].max(axis=(0, 2))
    baseE = np.concatenate([[0], np.cumsum(CEa)])
    baseO = np.concatenate([[0], np.cumsum(COa)])

    # idx arrays [core, sum(C)*128] flat (c*128+p within each tile segment)
    flatE = np.full((NC_N, int(CEa.sum()) * 128), DUM_E, dtype=np.int16)
    flatO = np.full((NC_N, int(COa.sum()) * 128), 0, dtype=np.int16)
    # dummy rows: E half -> core7 slot 3135 = row 7*3136+3135 = 25087
    #             O half -> core7 slot 6271 = row 25087
    flatE[:] = HROWS - 1
    flatO[:] = HROWS - 1
    e_sel = egrp
    posE = baseE[etile[e_sel]] * 128 + seq[e_sel] * 128 + elane[e_sel]
    flatE[ecore[e_sel], posE] = row_half[src[e_sel]].astype(np.int16)
    o_sel = ~egrp
    posO = baseO[etile[o_sel]] * 128 + seq[o_sel] * 128 + elane[o_sel]
    flatO[ecore[o_sel], posO] = row_half[src[o_sel]].astype(np.int16)

    idxE, idxO = [], []
    for k in range(NC_N):
        idxE.append(np.ascontiguousarray(
            np.tile(flatE[k].reshape(-1, 16).T, (8, 1))))
        idxO.append(np.ascontiguousarray(
            np.tile(flatO[k].reshape(-1, 16).T, (8, 1))))

    return {
        "CEa": CEa.astype(np.int64), "COa": COa.astype(np.int64),
        "node_core": node_core, "slot": slot,
        "idxE": idxE, "idxO": idxO,
        "key": (tuple(int(x) for x in CEa), tuple(int(x) for x in COa)),
    }


def _fold_w(W, al, ar):
    """Wc = [Wl | W | Wr]: el = h@Wl, ft = h@W, er = h@Wr."""
    Din = W.shape[0]
    H, C = al.shape
    W3 = W.reshape(Din, H, C)
    Wl = np.einsum("dhc,hc->dh", W3, al)
    Wr = np.einsum("dhc,hc->dh", W3, ar)
    return np.concatenate([Wl, W, Wr], axis=1).astype(BF16)


# ----------------------------------------------------------------------------
# Device program
# ----------------------------------------------------------------------------

def _build_program(CEa, COa):
    import concourse.bass as bass
    import concourse.mybir as mybir
    import concourse.tile as tile
    from concourse import bacc
    from concourse.masks import make_identity

    f32 = mybir.dt.float32
    bf16 = mybir.dt.bfloat16
    i16 = mybir.dt.int16
    Alu = mybir.AluOpType
    Act = mybir.ActivationFunctionType

    CEmax, COmax = int(max(CEa)), int(max(COa))
    CTmax = CEmax + COmax
    baseE = np.concatenate([[0], np.cumsum(CEa)]).astype(int)
    baseO = np.concatenate([[0], np.cumsum(COa)]).astype(int)
    NE, NO = int(baseE[-1]), int(baseO[-1])

    nc = bacc.Bacc("TRN2", target_bir_lowering=False, debug=False,
                   num_devices=NC_N, num_swdge_queues=4)

    # ---- I/O ----
    xT = nc.dram_tensor("xT", [128, SLOTS], bf16, kind="ExternalInput")
    wc0 = nc.dram_tensor("wc0", [128, 272], bf16, kind="ExternalInput")
    wc1 = nc.dram_tensor("wc1", [256, 272], bf16, kind="ExternalInput")
    wc2 = nc.dram_tensor("wc2", [256, 34], bf16, kind="ExternalInput")
    idxE = nc.dram_tensor("idxE", [128, NE * 8], i16, kind="ExternalInput")
    idxO = nc.dram_tensor("idxO", [128, NO * 8], i16, kind="ExternalInput")
    out_d = nc.dram_tensor("out", [SLOTS, 32], f32, kind="ExternalOutput")

    # ---- internal DRAM ----
    Ts0 = nc.dram_tensor("Ts0", [SLOTS, ROW01], bf16)
    Ts1 = nc.dram_tensor("Ts1", [SLOTS, ROW01], bf16)
    Ts2 = nc.dram_tensor("Ts2", [SLOTS, ROW2], bf16)
    TfE0 = nc.dram_tensor("TfE0", [HROWS, ROW01], bf16, addr_space="Shared")
    TfO0 = nc.dram_tensor("TfO0", [HROWS, ROW01], bf16, addr_space="Shared")
    TfE1 = nc.dram_tensor("TfE1", [HROWS, ROW01], bf16, addr_space="Shared")
    TfO1 = nc.dram_tensor("TfO1", [HROWS, ROW01], bf16, addr_space="Shared")
    TfE2 = nc.dram_tensor("TfE2", [HROWS, ROW2], bf16, addr_space="Shared")
    TfO2 = nc.dram_tensor("TfO2", [HROWS, ROW2], bf16, addr_space="Shared")

    def bc(ap, dims):
        return bass.AP(ap.tensor, ap.offset, [ap.ap[0]] + dims)

    with tile.TileContext(nc) as tc:
        import contextlib
        ctx = contextlib.ExitStack()
        with ctx:
            consts = ctx.enter_context(tc.tile_pool(name="consts", bufs=1))
            persist = ctx.enter_context(tc.tile_pool(name="persist", bufs=1))
            gpool = ctx.enter_context(tc.tile_pool(name="gather", bufs=2))
            mpool = ctx.enter_context(tc.tile_pool(name="msg", bufs=2))
            spool = ctx.enter_context(tc.tile_pool(name="small", bufs=3))
            tpool = ctx.enter_context(tc.tile_pool(name="tbuild", bufs=2))
            pp_agg = ctx.enter_context(tc.tile_pool(name="ps_agg", bufs=2, space="PSUM"))
            pp_tb = ctx.enter_context(tc.tile_pool(name="ps_tb", bufs=2, space="PSUM"))
            pp_tp = ctx.enter_context(tc.tile_pool(name="ps_tp", bufs=2, space="PSUM"))

            # ---- constants ----
            ident = consts.tile([128, 128], bf16)
            make_identity(nc, ident[:])
            zer_f32 = consts.tile([128, 1], f32)
            nc.gpsimd.memset(zer_f32[:], 0)
            eps_f32 = consts.tile([128, 1], f32)
            nc.gpsimd.memset(eps_f32[:], 1e-30)
            dum_row = consts.tile([1, ROW01], bf16)
            nc.gpsimd.memset(dum_row[:], 0)
            nc.gpsimd.memset(dum_row[0:1, 0:16].bitcast(f32), -1e4)

            idxE_sb = persist.tile([128, NE * 8], i16)
            nc.sync.dma_start(out=idxE_sb[:], in_=idxE[:, :])
            idxO_sb = persist.tile([128, NO * 8], i16)
            nc.sync.dma_start(out=idxO_sb[:], in_=idxO[:, :])
            xT_sb = persist.tile([128, SLOTS], bf16)
            nc.sync.dma_start(out=xT_sb[:], in_=xT[:, :])
            wc0_sb = persist.tile([128, 272], bf16)
            nc.sync.dma_start(out=wc0_sb[:], in_=wc0[:, :])
            wc1_sb = persist.tile([128, 2, 272], bf16)
            nc.sync.dma_start(out=wc1_sb[:, 0, :], in_=wc1[0:128, :])
            nc.sync.dma_start(out=wc1_sb[:, 1, :], in_=wc1[128:256, :])
            wc2_sb = persist.tile([128, 2, 34], bf16)
            nc.sync.dma_start(out=wc2_sb[:, 0, :], in_=wc2[0:128, :])
            nc.sync.dma_start(out=wc2_sb[:, 1, :], in_=wc2[128:256, :])

            er_bufs = [persist.tile([128, NT, 8], f32, tag=f"er{i}",
                                    name=f"er{i}")
                       for i in range(2)]

            qn = [0]

            def next_q():
                qn[0] = (qn[0] + 1) % 4
                return qn[0]

            def write_dummy_rows():
                for Ts, ROW in ((Ts0, ROW01), (Ts1, ROW01), (Ts2, ROW2)):
                    for r in (DUM_E, DUM_O):
                        nc.sync.dma_start(out=Ts[r:r + 1, :],
                                          in_=dum_row[0:1, 0:ROW])

            def build_epilogue(ps, t, layer, er_nxt, Ts):
                """ps [128, 2*ELW? ...] -> Ts row tile + er_nxt."""
                ELW, FT = (8, 256) if layer < 2 else (1, 32)
                ROW = ROW01 if layer < 2 else ROW2
                tsb = tpool.tile([128, ROW], bf16, tag=f"tsb{ROW}")
                nc.vector.tensor_copy(out=tsb[:, 0:2 * ELW].bitcast(f32),
                                      in_=ps[:, 0:ELW])
                nc.scalar.activation(out=tsb[:, 2 * ELW:2 * ELW + FT],
                                     in_=ps[:, ELW:ELW + FT], func=Act.Copy)
                nc.vector.tensor_copy(out=er_nxt[:, t, 0:ELW],
                                      in_=ps[:, ELW + FT:ELW + FT + ELW])
                # tiles holding the dummy rows skip them (written once at start)
                if t == DUM_E // 128:
                    pl = DUM_E % 128
                    nc.sync.dma_start(out=Ts[t * 128:t * 128 + pl, :],
                                      in_=tsb[0:pl, :])
                    nc.sync.dma_start(out=Ts[t * 128 + pl + 1:(t + 1) * 128, :],
                                      in_=tsb[pl + 1:128, :])
                elif t == DUM_O // 128:
                    pl = DUM_O % 128
                    nc.sync.dma_start(out=Ts[t * 128:t * 128 + pl, :],
                                      in_=tsb[0:pl, :])
                else:
                    nc.sync.dma_start(out=Ts[t * 128:(t + 1) * 128, :],
                                      in_=tsb[:])

            def build0():
                er_nxt = er_bufs[0]
                for t in range(NT):
                    ps = pp_tb.tile([128, 272], f32, tag="tb", space="PSUM")
                    nc.tensor.matmul(out=ps[:],
                                     lhsT=xT_sb[:, t * 128:(t + 1) * 128],
                                     rhs=wc0_sb[:], start=True, stop=True)
                    build_epilogue(ps, t, 0, er_nxt, Ts0)

            def allgather(Ts, TfE, TfO, ROW):
                nc.gpsimd.collective_compute(
                    "AllGather", Alu.bypass,
                    replica_groups=[list(range(NC_N))],
                    ins=[Ts[0:HSLOT, :]], outs=[TfE[:, :]])
                nc.gpsimd.collective_compute(
                    "AllGather", Alu.bypass,
                    replica_groups=[list(range(NC_N))],
                    ins=[Ts[HSLOT:SLOTS, :]], outs=[TfO[:, :]])

            def edge_phase(layer, TfE_l, TfO_l, Ts_nxt):
                if layer < 2:
                    ELW, FT, ROW = 8, 256, ROW01
                else:
                    ELW, FT, ROW = 1, 32, ROW2
                H = ELW
                er_cur = er_bufs[layer % 2]
                er_nxt = er_bufs[(layer + 1) % 2]
                wnxt = wc1_sb if layer == 0 else wc2_sb
                for t in range(NT):
                    CE, CO = int(CEa[t]), int(COa[t])
                    CT = CE + CO
                    gE = gpool.tile([128, CEmax, ROW], bf16, tag=f"gE{ROW}")
                    gO = gpool.tile([128, COmax, ROW], bf16, tag=f"gO{ROW}")
                    nc.gpsimd.dma_gather(
                        out_ap=gE[:, 0:CE, :], in_ap=TfE_l[:, :],
                        idxs_ap=idxE_sb[:, baseE[t] * 8:(baseE[t] + CE) * 8],
                        num_idxs=CE * 128, num_idxs_reg=CE * 128,
                        elem_size=ROW, single_packet=False, queue_num=next_q())
                    nc.gpsimd.dma_gather(
                        out_ap=gO[:, 0:CO, :], in_ap=TfO_l[:, :],
                        idxs_ap=idxO_sb[:, baseO[t] * 8:(baseO[t] + CO) * 8],
                        num_idxs=CO * 128, num_idxs_reg=CO * 128,
                        elem_size=ROW, single_packet=False, queue_num=next_q())

                    # z[p, h, c] = el + er ; h-major (packed stride CT) so the
                    # denominator is an innermost-axis reduce
                    z = spool.tile([128, 8 * CTmax], bf16, tag="z")
                    for g, c0, C in ((gE, 0, CE), (gO, CE, CO)):
                        nc.vector.tensor_tensor(
                            out=bc(z[:, c0:c0 + C], [[CT, H], [1, C]]),
                            in0=bc(g[:, 0:C, 0:2 * ELW].bitcast(f32),
                                   [[1, H], [ROW // 2, C]]),
                            in1=bc(er_cur[:, t, 0:H], [[1, H], [0, C]]),
                            op=Alu.add)
                    zf = z[:, 0:H * CT]
                    zs = spool.tile([128, 8 * CTmax], bf16, tag="zs")
                    nc.vector.scalar_tensor_tensor(
                        out=zs[:, 0:H * CT], in0=zf, scalar=NEG_SLOPE,
                        in1=zf, op0=Alu.mult, op1=Alu.max)
                    p = spool.tile([128, 8 * CTmax], bf16, tag="p")
                    nc.scalar.activation(
                        out=p[:, 0:H * CT], in_=zs[:, 0:H * CT], func=Act.Exp)

                    s_sb = spool.tile([128, 8], f32, tag="s")
                    nc.vector.tensor_reduce(
                        out=s_sb[:, 0:H], in_=bc(p[:], [[CT, H], [1, CT]]),
                        axis=mybir.AxisListType.X, op=Alu.add)
                    nc.vector.tensor_tensor(
                        out=s_sb[:, 0:H], in0=s_sb[:, 0:H],
                        in1=bc(eps_f32[:, 0:1], [[0, H]]), op=Alu.add)
                    rs = spool.tile([128, 8], f32, tag="rs")
                    nc.vector.reciprocal(out=rs[:, 0:H], in_=s_sb[:, 0:H])

                    msg = mpool.tile([128, CTmax, FT], bf16, tag=f"msg{FT}")
                    for g, c0, C in ((gE, 0, CE), (gO, CE, CO)):
                        nc.vector.tensor_tensor(
                            out=bc(msg[:, c0:c0 + C, :],
                                   [[FT, C], [32, H], [1, 32]]),
                            in0=bc(g[:, 0:C, 2 * ELW:2 * ELW + FT],
                                   [[ROW, C], [32, H], [1, 32]]),
                            in1=bc(p[:, c0:c0 + C], [[1, C], [CT, H], [0, 32]]),
                            op=Alu.mult)

                    agg = pp_agg.tile([128, 256], f32, tag="agg", space="PSUM")
                    for c in range(CT):
                        nc.tensor.matmul(out=agg[:, 0:FT], lhsT=ident[:],
                                         rhs=msg[:, c, :],
                                         start=(c == 0), stop=(c == CT - 1))

                    if layer < 2:
                        hrelu = spool.tile([128, 256], bf16, tag="hrelu")
                        nc.scalar.activation(out=hrelu[:], in_=agg[:, 0:256],
                                             func=Act.Relu)
                        h_t = spool.tile([128, 256], bf16, tag="ht")
                        nc.vector.tensor_tensor(
                            out=bc(h_t[:], [[32, H], [1, 32]]),
                            in0=bc(hrelu[:], [[32, H], [1, 32]]),
                            in1=bc(rs[:], [[1, H], [0, 32]]),
                            op=Alu.mult)
                        hTt = tpool.tile([128, 2, 128], bf16, tag="hTt")
                        for b in range(2):
                            tp = pp_tp.tile([128, 128], bf16, tag="tp",
                                            space="PSUM")
                            nc.tensor.transpose(
                                out=tp[:], in_=h_t[:, b * 128:(b + 1) * 128],
                                identity=ident[:])
                            nc.vector.tensor_copy(out=hTt[:, b, :], in_=tp[:])
                        ncols = 272 if layer == 0 else 34
                        ps2 = pp_tb.tile([128, 272], f32, tag="tb",
                                         space="PSUM")
                        for kb in range(2):
                            nc.tensor.matmul(out=ps2[:, 0:ncols],
                                             lhsT=hTt[:, kb, :],
                                             rhs=wnxt[:, kb, :],
                                             start=(kb == 0), stop=(kb == 1))
                        build_epilogue(ps2, t, layer + 1, er_nxt, Ts_nxt)
                    else:
                        o1 = spool.tile([128, 32], f32, tag="o1")
                        nc.scalar.activation(out=o1[:], in_=agg[:, 0:32],
                                             func=Act.Relu, scale=rs[:, 0:1])
                        nc.scalar.activation(out=o1[:], in_=o1[:], func=Act.Exp)
                        ssum = spool.tile([128, 1], f32, tag="ssum")
                        nc.vector.tensor_reduce(out=ssum[:], in_=o1[:],
                                                axis=mybir.AxisListType.X,
                                                op=Alu.add)
                        rr = spool.tile([128, 1], f32, tag="rr")
                        nc.vector.reciprocal(out=rr[:], in_=ssum[:])
                        ofin = spool.tile([128, 32], f32, tag="ofin")
                        nc.vector.tensor_tensor(out=ofin[:], in0=o1[:],
                                                in1=bc(rr[:], [[0, 32]]),
                                                op=Alu.mult)
                        nc.sync.dma_start(out=out_d[t * 128:(t + 1) * 128, :],
                                          in_=ofin[:])

            # ================= schedule =================
            write_dummy_rows()
            build0()
            allgather(Ts0, TfE0, TfO0, ROW01)
            edge_phase(0, TfE0, TfO0, Ts1)
            allgather(Ts1, TfE1, TfO1, ROW01)
            edge_phase(1, TfE1, TfO1, Ts2)
            allgather(Ts2, TfE2, TfO2, ROW2)
            edge_phase(2, TfE2, TfO2, None)

    nc.compile()
    return nc


# ----------------------------------------------------------------------------
# Entry point
# ----------------------------------------------------------------------------

def kernel(x, src, dst, W0, al0, ar0, b0, W1, al1, ar1, b1, W2, al2, ar2, b2):
    from concourse.bass_utils import run_bass_kernel_spmd

    x = np.asarray(x, dtype=np.float32)
    g = _prep_graph(src, dst)

    key = ("prog", g["key"])
    if key not in _CACHE:
        _CACHE[key] = _build_program(np.array(g["key"][0]),
                                     np.array(g["key"][1]))
    nc = _CACHE[key]

    wc0 = _fold_w(np.asarray(W0, np.float32), np.asarray(al0, np.float32),
                  np.asarray(ar0, np.float32))
    wc1 = _fold_w(np.asarray(W1, np.float32), np.asarray(al1, np.float32),
                  np.asarray(ar1, np.float32))
    wc2 = _fold_w(np.asarray(W2, np.float32), np.asarray(al2, np.float32),
                  np.asarray(ar2, np.float32))

    node_core, slot = g["node_core"], g["slot"]
    in_maps = []
    for k in range(NC_N):
        nodes = np.where(node_core == k)[0]
        xTk = np.zeros((128, SLOTS), dtype=BF16)
        xTk[:, slot[nodes]] = x[nodes].T.astype(BF16)
        in_maps.append({
            "xT": np.ascontiguousarray(xTk),
            "wc0": wc0, "wc1": wc1, "wc2": wc2,
            "idxE": g["idxE"][k], "idxO": g["idxO"][k],
        })

    global _last_in_maps
    _last_in_maps = in_maps
    res = run_bass_kernel_spmd(nc, in_maps, core_ids=list(range(NC_N)))

    out = np.empty((N_NODES, 32), dtype=np.float32)
    for k in range(NC_N):
        nodes = np.where(node_core == k)[0]
        out[nodes] = res.results[k]["out"][slot[nodes]]
    return out
